# revision 1
# baseline (speedup 1.0000x reference)
"""Multi-head causal attention with RoPE on 8 TRN2 NeuronCores.

Problem: x[2,2048,2048] @ {Wq,Wk,Wv}ᵀ -> 16-head causal attention with RoPE
-> @ Woᵀ.  Sharding: core i handles batch i//4 and head-group i%4 (4 heads,
512 of the 2048 projection channels).  Wq/Wk/Wv are row-sliced, Wo is
column-sliced; each core emits a partial yᵀ and the host sums the 4 partials
per batch (the tensor-parallel all-reduce done at unshard time).

Device-side layout choices (all matmul operands bf16, fp32 PSUM accumulate):
  - host passes xᵀ[h,s] and Wᵀ[h,o] so every matmul contracts over the
    partition dim with zero on-chip transposes
  - scores are computed transposed, Sᵀ[k,q] = Kᵀ-chunkᵀ @ Qᵀ, so the exp'd
    attention chunk is directly the lhsT/rhs the PV matmul needs and softmax's
    denominator is a ones-vector matmul on the PE (sum over partitions)
  - no max-subtraction: scores are ~N(0,1) after the 1/sqrt(128) scale (fused
    into the ACT exp), so exp can't overflow fp32
  - attention output accumulates as outᵀ[d,q], which is exactly the lhsT of
    the output projection; y leaves the chip transposed and the host fixes it
"""

import numpy as np
import ml_dtypes

import concourse.bass as bass
import concourse.tile as tile
import concourse.mybir as mybir
from concourse import bacc
from concourse.bass import ts
from concourse.bass_utils import run_bass_kernel_spmd

B, S, H = 2, 2048, 2048
HEADS, HD = 16, 128
NCORES = 8
GH = 4                 # heads per core
GO = GH * HD           # 512 projection channels per core
P = 128
SB = 512               # token-block (free dim of most matmuls)
NSB = S // SB          # 4
HC = H // P            # 16 contraction chunks of the hidden dim
NKC = S // P           # 16 key-token chunks
MASKW = 896            # staircase mask width: 384 + 512
SCALE = float(HD) ** -0.5

BF16 = mybir.dt.bfloat16
F32 = mybir.dt.float32
EXP = mybir.ActivationFunctionType.Exp

_built = {}


def _build():
    nc = bacc.Bacc(trn_type="TRN2")

    xt = nc.dram_tensor("xt", [H, S], BF16, kind="ExternalInput")
    wqt = nc.dram_tensor("wqt", [H, GO], BF16, kind="ExternalInput")
    wkt = nc.dram_tensor("wkt", [H, GO], BF16, kind="ExternalInput")
    wvt = nc.dram_tensor("wvt", [H, GO], BF16, kind="ExternalInput")
    wot = nc.dram_tensor("wot", [GO, H], BF16, kind="ExternalInput")
    cost = nc.dram_tensor("cost", [P, S], BF16, kind="ExternalInput")
    sint = nc.dram_tensor("sint", [P, S], BF16, kind="ExternalInput")
    # trimm[a, b] = -1e30 where b > a else 0; iden = identity.  The causal
    # mask is applied on the PE: psum[p, f] += trimm^T = -1e30 where p > f.
    trimm = nc.dram_tensor("trimm", [P, P], BF16, kind="ExternalInput")
    iden = nc.dram_tensor("iden", [P, P], BF16, kind="ExternalInput")
    yt = nc.dram_tensor("yt", [H, S], F32, kind="ExternalOutput")

    xt_r = xt[:].rearrange("(hc p) s -> p hc s", p=P)
    yt_r = yt[:].rearrange("(t p) s -> p t s", p=P)

    with tile.TileContext(nc) as tc:
        with (
            tc.tile_pool(name="const", bufs=1) as const,
            tc.tile_pool(name="xstream", bufs=2) as xpool,
            tc.tile_pool(name="rope", bufs=3) as rpool,
            tc.tile_pool(name="attn", bufs=8) as apool,
            tc.tile_pool(name="soft", bufs=2) as spool,
            tc.tile_pool(name="yout", bufs=3) as ypool,
            tc.tile_pool(name="pacc", bufs=2, space="PSUM") as pacc,
            tc.tile_pool(name="pscore", bufs=2, space="PSUM") as pscore,
            tc.tile_pool(name="pout", bufs=1, space="PSUM") as pout,
            tc.tile_pool(name="pden", bufs=1, space="PSUM") as pden,
        ):
            pproj = pacc
            py = pacc
            # ---- constants / persistent tensors ----
            # DMA issue order matters for the startup critical path: the first
            # projection matmul chain consumes xb(0) and w_q chunk-by-chunk,
            # so issue those interleaved in small pieces.
            xb0 = xpool.tile([P, HC, SB], BF16, tag="xb")
            w_q = const.tile([P, HC, GO], BF16, tag="wq")
            xt0 = xt_r[:, :, ts(0, SB)]
            wq_r = wqt[:].rearrange("(hc p) o -> p hc o", p=P)
            for qc in range(8):
                nc.sync.dma_start(xb0[:, ts(qc, 2), :], xt0[:, ts(qc, 2), :])
                nc.sync.dma_start(w_q[:, ts(qc, 2), :], wq_r[:, ts(qc, 2), :])
            w_k = const.tile([P, HC, GO], BF16, tag="wk")
            nc.sync.dma_start(w_k[:], wkt[:].rearrange("(hc p) o -> p hc o", p=P))
            cos_t = const.tile([P, S], BF16, tag="cos")
            nc.sync.dma_start(cos_t[:], cost[:])
            sin_t = const.tile([P, S], BF16, tag="sin")
            nc.sync.dma_start(sin_t[:], sint[:])
            w_v = const.tile([P, HC, GO], BF16, tag="wv")
            nc.sync.dma_start(w_v[:], wvt[:].rearrange("(hc p) o -> p hc o", p=P))
            xbs = [xb0]
            xb1 = xpool.tile([P, HC, SB], BF16, tag="xb")
            nc.sync.dma_start(xb1[:], xt_r[:, :, ts(1, SB)])
            xbs.append(xb1)
            w_o = const.tile([P, GH, H], BF16, tag="wo")
            nc.sync.dma_start(w_o[:], wot[:].rearrange("(oc p) n -> p oc n", p=P))
            tri_t = const.tile([P, P], BF16, tag="tri")
            nc.sync.dma_start(tri_t[:], trimm[:])
            id_t = const.tile([P, P], BF16, tag="iden")
            nc.sync.dma_start(id_t[:], iden[:])
            ones_t = const.tile([P, 1], BF16, tag="ones")
            nc.gpsimd.memset(ones_t[:], 1.0)

            q_t = const.tile([P, GH, S], BF16, tag="qt")
            k_t = const.tile([P, GH, S], BF16, tag="kt")
            v_t = const.tile([P, NKC, GO], BF16, tag="vt")
            out_t = const.tile([P, GH, S], BF16, tag="ot")

            # ---- emission generators (interleaved to keep the in-order PE
            # queue dense while ACT/DVE run dependent work) ----

            def proj_sb(sb, xb):
                """One token-block of Q/K (with RoPE) and V projections.
                Yields after each 16-matmul chain (~3.4us of PE work)."""
                for w_t, dest in ((w_q, q_t), (w_k, k_t)):
                    for h in range(GH):
                        ps = pproj.tile([P, SB], F32, tag="pp")
                        for hc in range(HC):
                            nc.tensor.matmul(
                                ps[:], w_t[:, hc, ts(h, P)], xb[:, hc, :],
                                start=(hc == 0), stop=(hc == HC - 1),
                            )
                        raw = dest[:, h, ts(sb, SB)]
                        nc.scalar.copy(raw, ps[:])
                        # RoPE: rot = raw*cos + shift(raw)*sin_signed
                        tmp = rpool.tile([P, SB], BF16, tag="sh")
                        nc.sync.dma_start(tmp[0:64, :], raw[64:128, :])
                        nc.sync.dma_start(tmp[64:128, :], raw[0:64, :])
                        tmp2 = rpool.tile([P, SB], BF16, tag="sp")
                        nc.vector.tensor_mul(tmp2[:], tmp[:], sin_t[:, ts(sb, SB)])
                        nc.vector.tensor_mul(raw, raw, cos_t[:, ts(sb, SB)])
                        nc.vector.tensor_add(raw, raw, tmp2[:])
                        yield
                for j in range(SB // P):
                    ps = pproj.tile([P, GO], F32, tag="pp")
                    for hc in range(HC):
                        nc.tensor.matmul(
                            ps[:], xb[:, hc, ts(j, P)], w_v[:, hc, :],
                            start=(hc == 0), stop=(hc == HC - 1),
                        )
                    nc.any.tensor_copy(v_t[:, sb * (SB // P) + j, :], ps[:])
                    yield

            def attn_block(b):
                """Attention for one 512-query block; yields per k-chunk.
                PV/den matmuls for chunk c are emitted one quantum late so
                the PE never waits on that chunk's exp."""
                nchunks = 4 * (b + 1)
                for h in range(GH):
                    po = pout.tile([P, SB], F32, tag="po")
                    pd = pden.tile([1, SB], F32, tag="pd")
                    pending = []

                    def flush(po=po, pd=pd, pending=pending):
                        if not pending:
                            return
                        c, qlo, n, at = pending.pop(0)
                        nc.tensor.matmul(
                            po[:, qlo:], v_t[:, c, ts(h, P)], at[:, :n],
                            start=(c == 0), stop=(c == nchunks - 1),
                        )
                        nc.tensor.matmul(
                            pd[:, qlo:], ones_t[:], at[:, :n],
                            start=(c == 0), stop=(c == nchunks - 1),
                        )

                    half = None  # (psum_pair_tile, first_chunk_idx)
                    for c in range(nchunks):
                        # causal column restriction: chunk c only reaches
                        # queries q >= 128*j (within this 512-block)
                        j = c - 4 * b
                        qlo = 128 * j if j > 0 else 0
                        n = SB - qlo
                        diag = j >= 0
                        if diag:
                            # single chunk in half 0 of a pair tile, with the
                            # -1e30 triangle accumulated on the PE
                            psc = pscore.tile([P, 2, SB], F32, tag="ps")
                            nc.tensor.matmul(
                                psc[:, 0, :n], k_t[:, h, ts(c, P)],
                                q_t[:, h, b * SB + qlo:(b + 1) * SB],
                                start=True, stop=False,
                            )
                            nc.tensor.matmul(
                                psc[:, 0, 0:128], tri_t[:], id_t[:],
                                start=False, stop=True,
                            )
                            at = apool.tile([P, 2, SB], BF16, tag="at")
                            nc.scalar.activation(
                                at[:, 0, :n], psc[:, 0, :n], EXP, scale=SCALE
                            )
                            pending.append((c, qlo, n, at[:, 0, :]))
                        elif half is None:
                            psc = pscore.tile([P, 2, SB], F32, tag="ps")
                            nc.tensor.matmul(
                                psc[:, 0, :], k_t[:, h, ts(c, P)],
                                q_t[:, h, ts(b, SB)],
                                start=True, stop=True,
                            )
                            half = (psc, c)
                        else:
                            psc, c0 = half
                            half = None
                            nc.tensor.matmul(
                                psc[:, 1, :], k_t[:, h, ts(c, P)],
                                q_t[:, h, ts(b, SB)],
                                start=True, stop=True,
                            )
                            at = apool.tile([P, 2, SB], BF16, tag="at")
                            nc.scalar.activation(
                                at[:, :, :], psc[:, :, :], EXP, scale=SCALE
                            )
                            pending.append((c0, 0, SB, at[:, 0, :]))
                            pending.append((c, 0, SB, at[:, 1, :]))
                        if len(pending) >= 3:
                            flush()
                        yield 2 if diag else 1
                    while pending:
                        flush()
                    rec1 = spool.tile([1, SB], F32, tag="den")
                    nc.vector.reciprocal_approx_fast(rec1[:], pd[:])
                    rec = spool.tile([P, SB], F32, tag="rec")
                    nc.gpsimd.partition_broadcast(rec[:], rec1[:])
                    nc.vector.tensor_mul(out_t[:, h, ts(b, SB)], po[:], rec[:])

            NT = H // P

            def outproj_block(ob):
                """Output projection of one query block; yields per matmul."""
                for nt in range(NT):
                    pyt = py.tile([P, SB], F32, tag="pp")
                    for oc in range(GH):
                        nc.tensor.matmul(
                            pyt[:], w_o[:, oc, ts(nt, P)],
                            out_t[:, oc, ts(ob, SB)],
                            start=(oc == 0), stop=(oc == GH - 1),
                        )
                    ysb = ypool.tile([P, SB], F32, tag="ysb")
                    nc.vector.tensor_copy(ysb[:], pyt[:])
                    nc.sync.dma_start(yt_r[:, nt, ts(ob, SB)], ysb[:])
                    yield

            def drain(gen):
                for _ in gen:
                    pass

            def chain2(*gens):
                for g in gens:
                    yield from g

            def interleave(primary, filler, fill_per_quantum, drain_rest=True):
                """Emit primary; after each primary quantum, emit
                fill_per_quantum (x quantum weight) quanta of filler."""
                acc = 0.0
                for w in primary:
                    acc += fill_per_quantum * (w or 1)
                    while acc >= 1.0:
                        if next(filler, StopIteration) is StopIteration:
                            acc = 0.0
                            break
                        acc -= 1.0
                if drain_rest:
                    drain(filler)

            # ---- drive ----
            # Phase 1 (PE-bound) hides the ACT-heavy attention of blocks 0+1;
            # attention of blocks 2+3 (ACT-heavy) hides outproj (PE-bound).
            drain(proj_sb(0, xbs[0]))

            def proj_rest():
                for sb in range(1, NSB):
                    if sb < 2:
                        xb = xbs[sb]
                    else:
                        xb = xpool.tile([P, HC, SB], BF16, tag="xb")
                        nc.sync.dma_start(xb[:], xt_r[:, :, ts(sb, SB)])
                    yield from proj_sb(sb, xb)

            interleave(proj_rest(), attn_block(0), 16 / 36)
            interleave(attn_block(1), outproj_block(0), 16 / 48)
            op1 = outproj_block(1)
            interleave(attn_block(2), op1, 12 / 64, drain_rest=False)
            interleave(attn_block(3), chain2(op1, outproj_block(2)), 20 / 80)
            drain(outproj_block(NSB - 1))

    nc.compile()
    return nc


def _get_nc():
    if "nc" not in _built:
        _built["nc"] = _build()
    return _built["nc"]


def _host_inputs(x, Wq, Wk, Wv, Wo):
    bf = ml_dtypes.bfloat16
    inv = 1.0 / (10000.0 ** (np.arange(0, HD, 2, dtype=np.float64) / HD))
    t = np.arange(S, dtype=np.float64)
    fr = np.outer(t, inv)                       # [S, 64]
    cos = np.cos(fr)
    sin = np.sin(fr)
    cosT = np.concatenate([cos, cos], axis=1).T.astype(bf)      # [128, S]
    sinT = np.concatenate([-sin, sin], axis=1).T.astype(bf)     # signed
    a = np.arange(P)
    trimm = np.where(a[None, :] > a[:, None], -1e30, 0.0).astype(bf)
    iden = np.eye(P).astype(bf)

    in_maps = []
    for core in range(NCORES):
        b, g = divmod(core, GH)
        sl = slice(GO * g, GO * (g + 1))
        in_maps.append({
            "xt": np.ascontiguousarray(x[b].T).astype(bf),
            "wqt": np.ascontiguousarray(Wq[sl, :].T).astype(bf),
            "wkt": np.ascontiguousarray(Wk[sl, :].T).astype(bf),
            "wvt": np.ascontiguousarray(Wv[sl, :].T).astype(bf),
            "wot": np.ascontiguousarray(Wo[:, sl].T).astype(bf),
            "cost": cosT.copy(),
            "sint": sinT.copy(),
            "trimm": trimm.copy(),
            "iden": iden.copy(),
        })
    return in_maps


def kernel(x, Wq, Wk, Wv, Wo, _trace=False):
    x = np.asarray(x, dtype=np.float32)
    Wq = np.asarray(Wq, dtype=np.float32)
    Wk = np.asarray(Wk, dtype=np.float32)
    Wv = np.asarray(Wv, dtype=np.float32)
    Wo = np.asarray(Wo, dtype=np.float32)

    nc = _get_nc()
    in_maps = _host_inputs(x, Wq, Wk, Wv, Wo)
    res = run_bass_kernel_spmd(
        nc, in_maps, core_ids=list(range(NCORES)), trace=_trace
    )
    if _trace:
        _built["last_result"] = res

    y = np.zeros((B, S, H), dtype=np.float32)
    for core in range(NCORES):
        b = core // GH
        y[b] += res.results[core]["yt"].T
    return y

